# revision 48
# baseline (speedup 1.0000x reference)
"""Bass/Trainium2 kernel for fused bilinear attention + softmax.

reference computation:
    pa = a @ Wa + ba                      (B, La, D)
    pb = b @ Wb + bb                      (B, Lb, D)
    scores = einsum('bid,bjd->bij', pa * w, pb) + wbias
    out = softmax(scores.reshape(B, La*Lb)).reshape(B, La, Lb)

Device strategy (8 NeuronCores, data-parallel over batch, 8 batches/core):
    Weight-only host folding:  M = (Wa*w) @ Wb.T,  u = (Wa*w)@bb,  v = (Wb*w)@ba
      scores[b,i,j] = a_i M b_j^T + (a_i.u) + (b_j.v) + const
    const (+wbias) is dropped: softmax over the flattened grid is shift-invariant.
    Host pre-transposes a,b to feature-major bf16 (aT, bT), so no on-device
    transposes are needed.  Per pair of batches (rhs free dim 512):
      TT   = M @ bT + u          64 bf16 matmuls (N=512) + ACT eviction w/ bias
      S    = aT^T @ TT + 1 (x) bu   36 bf16 matmuls (N=256), bu via K=1 matmul
      softmax: ACT exp -> DVE row-sum -> PE ones-matmul cross-partition sum
               -> DVE reciprocal -> PE broadcast matmul -> DVE scale
"""

import numpy as np
import ml_dtypes

import concourse.bass as bass
import concourse.bacc as bacc
import concourse.mybir as mybir
import concourse.tile as tile
from concourse.bass_utils import run_bass_kernel_spmd

BF16 = ml_dtypes.bfloat16

N_CORES = 8
B, L, K = 64, 256, 1024          # batch, seq len (La=Lb), feature dim (IN_A=IN_B)
BPC = B // N_CORES               # batches per core
G = BPC // 2                     # batch-pair groups per core
KC = K // 128                    # feature chunks of 128
F32 = mybir.dt.float32
DBF = mybir.dt.bfloat16
Act = mybir.ActivationFunctionType


def _build_program():
    # Bacc (not raw Bass): its compile() legalizes multi-wait instructions
    # (TRN2 allows at most one sync wait per instruction).
    nc = bacc.Bacc("TRN2", debug=False, target_bir_lowering=False)

    at = nc.dram_tensor("at", [G, K, 2 * L], DBF, kind="ExternalInput")
    bt = nc.dram_tensor("bt", [G, K, 2 * L], DBF, kind="ExternalInput")
    # M^T in m-major blocks: mt[m, p, l_chunk, ki] = M.T[l_chunk*128+p, m*128+ki]
    mt = nc.dram_tensor("mt", [KC, 128, KC, 128], DBF, kind="ExternalInput")
    u = nc.dram_tensor("u", [K], F32, kind="ExternalInput")
    v = nc.dram_tensor("v", [K], F32, kind="ExternalInput")
    probs = nc.dram_tensor("probs", [BPC, L, L], F32, kind="ExternalOutput")

    with tile.TileContext(nc) as tc:
        with (
            tc.tile_pool(name="consts", bufs=1) as consts,
            tc.tile_pool(name="inp", bufs=2) as in_pool,
            tc.tile_pool(name="tt", bufs=10) as tt_pool,
            tc.tile_pool(name="sm", bufs=4) as sm_pool,
            tc.tile_pool(name="small", bufs=4) as small,
            tc.tile_pool(name="ps_tt", bufs=5, space="PSUM") as ps_tt,
            tc.tile_pool(name="ps_sc", bufs=2, space="PSUM") as ps_sc,
            tc.tile_pool(name="ps_bu", bufs=1, space="PSUM") as ps_bu,
        ):
            # ---- constants (chunked DMAs so PE can start on chunk 0 early) ----
            u_sb = consts.tile([128, KC], F32)              # u[c*128+p] at [p, c]
            nc.sync.dma_start(out=u_sb, in_=u[:].rearrange("(c p) -> p c", p=128))
            v_sb = consts.tile([128, KC], F32)
            nc.sync.dma_start(out=v_sb, in_=v[:].rearrange("(c p) -> p c", p=128))
            mt_sb = consts.tile([128, KC, KC, 128], DBF)    # [l_in, m, l_chunk, ki]
            ones_col_f32 = consts.tile([128, 1], F32)
            nc.vector.memset(ones_col_f32, 1.0)
            ones_row_f32 = consts.tile([1, 128], F32)
            nc.vector.memset(ones_row_f32, 1.0)
            ones_row_bf = consts.tile([1, 128], DBF)
            nc.vector.memset(ones_row_bf, 1.0)
            ones_col_bf = consts.tile([128, 1], DBF)
            nc.vector.memset(ones_col_bf, 1.0)

            from concourse import library_config
            nc.gpsimd.load_library(library_config.attnmlp)

            # PE warm-up: dummy matmuls while the first DMAs land, so the HAM
            # clock gate is already released when real matmuls start.
            warm_sb = consts.tile([128, 2 * L], DBF)
            nc.vector.memset(warm_sb, 0.0)
            warm_ps = ps_bu.tile([128, 2 * L], F32, tag="bu_ps")
            for i in range(5):
                nc.tensor.matmul(
                    warm_ps, warm_sb[:, 0:128], warm_sb,
                    start=(i == 0), stop=(i == 4),
                )

            for g in range(G):
                bt_sb = in_pool.tile([128, KC, 2 * L], DBF, tag="bt")
                for l in range(KC):
                    if g == 0:
                        # interleave M^T m-blocks with the first group's bT so
                        # the TT accumulation can start as blocks arrive
                        # (SWDGE ring: parallel to the HWDGE input loads)
                        nc.sync.dma_start(out=mt_sb[:, l], in_=mt[l])
                    nc.sync.dma_start(
                        out=bt_sb[:, l, :], in_=bt[g, l * 128 : (l + 1) * 128, :]
                    )
                at_sb = in_pool.tile([128, KC, 2 * L], DBF, tag="at")
                for l in range(KC):
                    nc.sync.dma_start(
                        out=at_sb[:, l, :], in_=at[g, l * 128 : (l + 1) * 128, :]
                    )

                # Phase 1: all 8 TT chunks (kept in SBUF; tt_pool holds them all)
                tt_chunks = []
                for m in range(KC):
                    # TT chunk m: rows k in [128m, 128m+128), all 512 cols
                    tt_ps = ps_tt.tile([128, 2 * L], F32, tag="tt_ps")
                    for l in range(KC):
                        nc.tensor.matmul(
                            tt_ps, mt_sb[:, m, l, :], bt_sb[:, l, :],
                            start=(l == 0), stop=(l == KC - 1),
                        )
                    tt_sb = tt_pool.tile([128, 2 * L], DBF, tag="tt")
                    # TT' = TT + u[chunk m] (folds the a.u rank-1 term); DVE
                    # (not ACT) so the scalar engine never swaps LUT tables.
                    nc.vector.tensor_scalar_add(tt_sb, tt_ps, u_sb[:, m : m + 1])
                    tt_chunks.append(tt_sb)
                    # one bu MAC step per chunk, AFTER the eviction in DVE
                    # order: evictions gate PSUM recycling, bu is slack work.
                    # bu[j] = sum_l v[l] * bT[l, j]
                    if m == 0:
                        bu_acc = tt_pool.tile([128, 2 * L], DBF, tag="buacc")
                        nc.vector.tensor_scalar_mul(
                            bu_acc, bt_sb[:, 0, :], v_sb[:, 0:1]
                        )
                    else:
                        nc.vector.scalar_tensor_tensor(
                            bu_acc, bt_sb[:, m, :], v_sb[:, m : m + 1], bu_acc,
                            op0=mybir.AluOpType.mult, op1=mybir.AluOpType.add,
                        )

                # reduce bu over partitions (one PE matmul) and stage as bf16
                bu_ps = ps_bu.tile([1, 2 * L], F32, tag="bu_ps")
                nc.tensor.matmul(
                    bu_ps, ones_col_bf, bu_acc, start=True, stop=True
                )
                bu_sb = small.tile([1, 2 * L], DBF, tag="bu")
                nc.vector.tensor_copy(out=bu_sb, in_=bu_ps)

                # Phase 2: scores per batch in ONE psum bank (sequential h
                # accumulation groups), then a single fused exp+rowsum.
                for q in range(2):
                    sc_ps = ps_sc.tile([128, 2 * L], F32, tag="sc")
                    for h in range(2):
                        for m in range(KC):
                            nc.tensor.matmul(
                                sc_ps[:, h * L : (h + 1) * L],
                                at_sb[:, m, q * L + h * 128 : q * L + h * 128 + 128],
                                tt_chunks[m][:, q * L : (q + 1) * L],
                                start=(m == 0), stop=False,
                            )
                        # inject bu (K=1 accumulate): S[i, j] += 1 * bu[j]
                        nc.tensor.matmul(
                            sc_ps[:, h * L : (h + 1) * L],
                            ones_row_bf, bu_sb[:, q * L : (q + 1) * L],
                            start=False, stop=True,
                        )

                    # ---- softmax over the whole (256, 256) grid per batch ----
                    exp_sb = sm_pool.tile([128, 2 * L], F32, tag="exp")
                    colsum = small.tile([128, 1], F32, tag="cs")
                    nc.scalar.activation(
                        exp_sb, sc_ps, Act.Exp, accum_out=colsum
                    )
                    # total over partitions, broadcast to all (GpSimd), recip
                    tot_col = small.tile([128, 1], F32, tag="totc")
                    nc.gpsimd.partition_all_reduce(
                        tot_col, colsum, channels=128,
                        reduce_op=bass.bass_isa.ReduceOp.add,
                    )
                    rcp_col = small.tile([128, 1], F32, tag="rcpc")
                    nc.vector.reciprocal(rcp_col, tot_col)
                    probs_sb = sm_pool.tile([128, 2 * L], F32, tag="probs")
                    for h in range(2):
                        # split by half so the first DMA overlaps the second mul
                        nc.vector.tensor_scalar_mul(
                            probs_sb[:, h * L : (h + 1) * L],
                            exp_sb[:, h * L : (h + 1) * L],
                            rcp_col,
                        )
                        nc.sync.dma_start(
                            out=probs[2 * g + q][h * 128 : (h + 1) * 128, :],
                            in_=probs_sb[:, h * L : (h + 1) * L],
                        )
    return nc


def _prep_host(a, b, Wa, ba, Wb, bb, w, wbias):
    """Weight folding (f64) + per-core feature-major bf16 shards."""
    Wa64 = Wa.astype(np.float64)
    Wb64 = Wb.astype(np.float64)
    w64 = w.astype(np.float64)
    M = (Wa64 * w64[None, :]) @ Wb64.T                  # (K, K)
    u_np = ((Wa64 * w64[None, :]) @ bb.astype(np.float64)).astype(np.float32)
    v_np = ((Wb64 * w64[None, :]) @ ba.astype(np.float64)).astype(np.float32)
    # m-major blocked M^T: mt[m, p, c, ki] = M.T[c*128+p, m*128+ki]
    mt_np = np.ascontiguousarray(
        M.T.astype(np.float32)
        .reshape(KC, 128, KC, 128)
        .transpose(2, 1, 0, 3)
    ).astype(BF16)

    def shard(x):
        # (BPC, L, K) -> (G, K, 2L) feature-major bf16, batch pairs side by side
        xt = x.transpose(0, 2, 1)                        # (BPC, K, L)
        xt = xt.reshape(G, 2, K, L).transpose(0, 2, 1, 3).reshape(G, K, 2 * L)
        return np.ascontiguousarray(xt).astype(BF16)

    in_maps = []
    for c in range(N_CORES):
        sl = slice(c * BPC, (c + 1) * BPC)
        in_maps.append(
            {
                "at": shard(a[sl]),
                "bt": shard(b[sl]),
                "mt": mt_np,
                "u": u_np,
                "v": v_np,
            }
        )
    return in_maps


def _run(inputs, trace=False):
    nc = _build_program()
    nc.compile()
    in_maps = _prep_host(**inputs)
    res = run_bass_kernel_spmd(
        nc, in_maps, core_ids=list(range(N_CORES)), trace=trace
    )
    out = np.concatenate([res.results[c]["probs"] for c in range(N_CORES)], axis=0)
    return out.astype(np.float32), res


def kernel(**inputs) -> np.ndarray:
    out, _ = _run(inputs, trace=False)
    return out


# revision 49
# speedup vs baseline: 1.0393x; 1.0393x over previous
"""Bass/Trainium2 kernel for fused bilinear attention + softmax.

reference computation:
    pa = a @ Wa + ba                      (B, La, D)
    pb = b @ Wb + bb                      (B, Lb, D)
    scores = einsum('bid,bjd->bij', pa * w, pb) + wbias
    out = softmax(scores.reshape(B, La*Lb)).reshape(B, La, Lb)

Device strategy (8 NeuronCores, data-parallel over batch, 8 batches/core):
    Weight-only host folding:  M = (Wa*w) @ Wb.T,  u = (Wa*w)@bb,  v = (Wb*w)@ba
      scores[b,i,j] = a_i M b_j^T + (a_i.u) + (b_j.v) + const
    const (+wbias) is dropped: softmax over the flattened grid is shift-invariant.
    Host pre-transposes a,b to feature-major bf16 (aT, bT), so no on-device
    transposes are needed.  Per pair of batches (rhs free dim 512):
      TT   = M @ bT + u       64 bf16 matmuls (N=512); DVE eviction adds u
      bu   = v . bT           DVE multiply-accumulate + one PE reduce matmul
      S    = aT^T @ TT + 1(x)bu  36 bf16 matmuls (N=256), bu via K=1 matmul
      softmax: fused ACT exp+rowsum (accum_out) -> GpSimd partition_all_reduce
               -> DVE reciprocal -> DVE scale -> DMA out
    PE warm-up matmuls run during the initial DMAs (HAM clock-gate release).
"""

import numpy as np
import ml_dtypes

import concourse.bass as bass
import concourse.bacc as bacc
import concourse.mybir as mybir
import concourse.tile as tile
from concourse.bass_utils import run_bass_kernel_spmd

BF16 = ml_dtypes.bfloat16

N_CORES = 8
B, L, K = 64, 256, 1024          # batch, seq len (La=Lb), feature dim (IN_A=IN_B)
BPC = B // N_CORES               # batches per core
G = BPC // 2                     # batch-pair groups per core
KC = K // 128                    # feature chunks of 128
F32 = mybir.dt.float32
DBF = mybir.dt.bfloat16
Act = mybir.ActivationFunctionType


def _build_program():
    # Bacc (not raw Bass): its compile() legalizes multi-wait instructions
    # (TRN2 allows at most one sync wait per instruction).
    nc = bacc.Bacc("TRN2", debug=False, target_bir_lowering=False)

    at = nc.dram_tensor("at", [G, K, 2 * L], DBF, kind="ExternalInput")
    bt = nc.dram_tensor("bt", [G, K, 2 * L], DBF, kind="ExternalInput")
    # M^T in m-major blocks: mt[m, p, l_chunk, ki] = M.T[l_chunk*128+p, m*128+ki]
    mt = nc.dram_tensor("mt", [KC, 128, KC, 128], DBF, kind="ExternalInput")
    u = nc.dram_tensor("u", [K], F32, kind="ExternalInput")
    v = nc.dram_tensor("v", [K], F32, kind="ExternalInput")
    probs = nc.dram_tensor("probs", [BPC, L, L], F32, kind="ExternalOutput")

    with tile.TileContext(nc) as tc:
        with (
            tc.tile_pool(name="consts", bufs=1) as consts,
            tc.tile_pool(name="inp", bufs=2) as in_pool,
            tc.tile_pool(name="tt", bufs=10) as tt_pool,
            tc.tile_pool(name="sm", bufs=4) as sm_pool,
            tc.tile_pool(name="small", bufs=4) as small,
            tc.tile_pool(name="ps_tt", bufs=5, space="PSUM") as ps_tt,
            tc.tile_pool(name="ps_sc", bufs=2, space="PSUM") as ps_sc,
            tc.tile_pool(name="ps_bu", bufs=1, space="PSUM") as ps_bu,
        ):
            # ---- constants (chunked DMAs so PE can start on chunk 0 early) ----
            u_sb = consts.tile([128, KC], F32)              # u[c*128+p] at [p, c]
            nc.sync.dma_start(out=u_sb, in_=u[:].rearrange("(c p) -> p c", p=128))
            v_sb = consts.tile([128, KC], F32)
            nc.sync.dma_start(out=v_sb, in_=v[:].rearrange("(c p) -> p c", p=128))
            mt_sb = consts.tile([128, KC, KC, 128], DBF)    # [l_in, m, l_chunk, ki]
            ones_col_f32 = consts.tile([128, 1], F32)
            nc.vector.memset(ones_col_f32, 1.0)
            ones_row_f32 = consts.tile([1, 128], F32)
            nc.vector.memset(ones_row_f32, 1.0)
            ones_row_bf = consts.tile([1, 128], DBF)
            nc.vector.memset(ones_row_bf, 1.0)
            ones_col_bf = consts.tile([128, 1], DBF)
            nc.vector.memset(ones_col_bf, 1.0)

            from concourse import library_config
            nc.gpsimd.load_library(library_config.attnmlp)

            # PE warm-up: dummy matmuls while the first DMAs land, so the HAM
            # clock gate is already released when real matmuls start.
            warm_sb = consts.tile([128, 2 * L], DBF)
            nc.vector.memset(warm_sb, 0.0)
            warm_ps = ps_bu.tile([128, 2 * L], F32, tag="bu_ps")
            for i in range(5):
                nc.tensor.matmul(
                    warm_ps, warm_sb[:, 0:128], warm_sb,
                    start=(i == 0), stop=(i == 4),
                )

            for g in range(G):
                bt_sb = in_pool.tile([128, KC, 2 * L], DBF, tag="bt")
                for l in range(KC):
                    if g == 0:
                        # interleave M^T m-blocks with the first group's bT so
                        # the TT accumulation can start as blocks arrive
                        nc.sync.dma_start(out=mt_sb[:, l], in_=mt[l])
                    nc.sync.dma_start(
                        out=bt_sb[:, l, :], in_=bt[g, l * 128 : (l + 1) * 128, :]
                    )
                at_sb = in_pool.tile([128, KC, 2 * L], DBF, tag="at")
                for l in range(KC):
                    nc.sync.dma_start(
                        out=at_sb[:, l, :], in_=at[g, l * 128 : (l + 1) * 128, :]
                    )

                # Phase 1: all 8 TT chunks (kept in SBUF; tt_pool holds them all)
                tt_chunks = []
                for m in range(KC):
                    # TT chunk m: rows k in [128m, 128m+128), all 512 cols
                    tt_ps = ps_tt.tile([128, 2 * L], F32, tag="tt_ps")
                    for l in range(KC):
                        nc.tensor.matmul(
                            tt_ps, mt_sb[:, m, l, :], bt_sb[:, l, :],
                            start=(l == 0), stop=(l == KC - 1),
                        )
                    tt_sb = tt_pool.tile([128, 2 * L], DBF, tag="tt")
                    # TT' = TT + u[chunk m] (folds the a.u rank-1 term); DVE
                    # (not ACT) so the scalar engine never swaps LUT tables.
                    nc.vector.tensor_scalar_add(tt_sb, tt_ps, u_sb[:, m : m + 1])
                    tt_chunks.append(tt_sb)
                    # one bu MAC step per chunk, AFTER the eviction in DVE
                    # order: evictions gate PSUM recycling, bu is slack work.
                    # bu[j] = sum_l v[l] * bT[l, j]
                    if m == 0:
                        bu_acc = tt_pool.tile([128, 2 * L], DBF, tag="buacc")
                        nc.vector.tensor_scalar_mul(
                            bu_acc, bt_sb[:, 0, :], v_sb[:, 0:1]
                        )
                    else:
                        nc.vector.scalar_tensor_tensor(
                            bu_acc, bt_sb[:, m, :], v_sb[:, m : m + 1], bu_acc,
                            op0=mybir.AluOpType.mult, op1=mybir.AluOpType.add,
                        )

                # reduce bu over partitions (one PE matmul) and stage as bf16
                bu_ps = ps_bu.tile([1, 2 * L], F32, tag="bu_ps")
                nc.tensor.matmul(
                    bu_ps, ones_col_bf, bu_acc, start=True, stop=True
                )
                bu_sb = small.tile([1, 2 * L], DBF, tag="bu")
                nc.vector.tensor_copy(out=bu_sb, in_=bu_ps)

                # Phase 2: scores per batch in ONE psum bank (sequential h
                # accumulation groups), then a single fused exp+rowsum.
                for q in range(2):
                    sc_ps = ps_sc.tile([128, 2 * L], F32, tag="sc")
                    for h in range(2):
                        for m in range(KC):
                            nc.tensor.matmul(
                                sc_ps[:, h * L : (h + 1) * L],
                                at_sb[:, m, q * L + h * 128 : q * L + h * 128 + 128],
                                tt_chunks[m][:, q * L : (q + 1) * L],
                                start=(m == 0), stop=False,
                            )
                        # inject bu (K=1 accumulate): S[i, j] += 1 * bu[j]
                        nc.tensor.matmul(
                            sc_ps[:, h * L : (h + 1) * L],
                            ones_row_bf, bu_sb[:, q * L : (q + 1) * L],
                            start=False, stop=True,
                        )

                    # ---- softmax over the whole (256, 256) grid per batch ----
                    exp_sb = sm_pool.tile([128, 2 * L], F32, tag="exp")
                    colsum = small.tile([128, 1], F32, tag="cs")
                    nc.scalar.activation(
                        exp_sb, sc_ps, Act.Exp, accum_out=colsum
                    )
                    # total over partitions, broadcast to all (GpSimd), recip
                    tot_col = small.tile([128, 1], F32, tag="totc")
                    nc.gpsimd.partition_all_reduce(
                        tot_col, colsum, channels=128,
                        reduce_op=bass.bass_isa.ReduceOp.add,
                    )
                    rcp_col = small.tile([128, 1], F32, tag="rcpc")
                    nc.vector.reciprocal(rcp_col, tot_col)
                    probs_sb = sm_pool.tile([128, 2 * L], F32, tag="probs")
                    for h in range(2):
                        # split by half so the first DMA overlaps the second mul
                        nc.vector.tensor_scalar_mul(
                            probs_sb[:, h * L : (h + 1) * L],
                            exp_sb[:, h * L : (h + 1) * L],
                            rcp_col,
                        )
                        nc.sync.dma_start(
                            out=probs[2 * g + q][h * 128 : (h + 1) * 128, :],
                            in_=probs_sb[:, h * L : (h + 1) * L],
                        )
    return nc


def _prep_host(a, b, Wa, ba, Wb, bb, w, wbias):
    """Weight folding (f64) + per-core feature-major bf16 shards."""
    Wa64 = Wa.astype(np.float64)
    Wb64 = Wb.astype(np.float64)
    w64 = w.astype(np.float64)
    M = (Wa64 * w64[None, :]) @ Wb64.T                  # (K, K)
    u_np = ((Wa64 * w64[None, :]) @ bb.astype(np.float64)).astype(np.float32)
    v_np = ((Wb64 * w64[None, :]) @ ba.astype(np.float64)).astype(np.float32)
    # m-major blocked M^T: mt[m, p, c, ki] = M.T[c*128+p, m*128+ki]
    mt_np = np.ascontiguousarray(
        M.T.astype(np.float32)
        .reshape(KC, 128, KC, 128)
        .transpose(2, 1, 0, 3)
    ).astype(BF16)

    def shard(x):
        # (BPC, L, K) -> (G, K, 2L) feature-major bf16, batch pairs side by side
        xt = x.transpose(0, 2, 1)                        # (BPC, K, L)
        xt = xt.reshape(G, 2, K, L).transpose(0, 2, 1, 3).reshape(G, K, 2 * L)
        return np.ascontiguousarray(xt).astype(BF16)

    in_maps = []
    for c in range(N_CORES):
        sl = slice(c * BPC, (c + 1) * BPC)
        in_maps.append(
            {
                "at": shard(a[sl]),
                "bt": shard(b[sl]),
                "mt": mt_np,
                "u": u_np,
                "v": v_np,
            }
        )
    return in_maps


def _run(inputs, trace=False):
    nc = _build_program()
    nc.compile()
    in_maps = _prep_host(**inputs)
    res = run_bass_kernel_spmd(
        nc, in_maps, core_ids=list(range(N_CORES)), trace=trace
    )
    out = np.concatenate([res.results[c]["probs"] for c in range(N_CORES)], axis=0)
    return out.astype(np.float32), res


def kernel(**inputs) -> np.ndarray:
    out, _ = _run(inputs, trace=False)
    return out
